# revision 2
# baseline (speedup 1.0000x reference)
"""Trainium2 Bass kernel for nn_BinaryDense: y = x @ binarize(w).T

x: [8192, 4096] f32, weight: [4096, 4096] f32 -> y: [8192, 4096] f32.

binarize(w) = +1 if fp32(w + 1.0) > 1.0 else -1, i.e. w > 2**-24 -> +1
(reference: round-half-even(clip((w+1)/2, 0, 1)) * 2 - 1 with H=1).

Strategy (8 cores), v5 — v4 architecture (PE transposes + pipelined fp16
AllGather, interleaved w-half/x-quarter prep, SWDGE stores) with the
prep critical path tightened:

  - all PE transposes run in fp16 (x is DVE-cast to fp16 before its
    transpose; fp32 transpose-mode is 2 cycles/row, fp16 is 1).
  - transposes write groups of four 128x128 tiles into one [128, 512]
    PSUM tile, drained by a SINGLE strided DVE copy (3D access pattern)
    instead of four small ones — 96 PSUM-drain copies total instead of
    384, removing DVE instruction-overhead pacing from the prep span.
  - w^T staging is one [128, 16K] slab (itl-blocked) so the grouped
    copies land with a strided AP; AllGather staging stores slice it.
  - matmul phase unchanged from v4: per 512-col o-tile, stream w^T
    i-tiles on alternating HWDGE queues, one fp16 matmul pass into fp32
    PSUM (8 banks = 8 row tiles), drain via DVE + SWDGE store.
"""

import numpy as np

import concourse.bass as bass
import concourse.tile as tile
from concourse import bacc, mybir
from concourse.bass_utils import run_bass_kernel_spmd
from concourse.masks import make_identity

N_CORES = 8
B = 1024            # rows of x per core
D = 4096            # in/out features
ISH = D // N_CORES  # 512, i-columns of w per core
BT = 128            # b tile (psum partition)
OT = 512            # o tile (psum free / one bank)
IT = 128            # contraction tile (partitions)
N_BT = B // BT      # 8
N_OT = D // OT      # 8
N_IT = D // IT      # 32
N_HALF = 4          # AllGather pipeline depth (o-range splits)
OH = D // N_HALF    # 1024 o-rows per collective
XQ = 4              # x i-quarters
QW = D // XQ        # 1024 i-cols per quarter
N_ITQ = N_IT // XQ  # 8 i-tiles per quarter
N_ITL = ISH // 128  # 4 i-tile rows of the w shard

F32 = mybir.dt.float32
F16 = mybir.dt.float16

BIN_THRESH = float(2.0 ** -24)

_CACHED = {}


def _build(repeat=1, include_ag=True, include_c=True, include_prep=True):
    nc = bacc.Bacc("TRN2", target_bir_lowering=False, debug=False,
                   num_devices=N_CORES)
    x = nc.dram_tensor("x", [B, D], F32, kind="ExternalInput").ap()
    # w columns shard: w[:, c*512:(c+1)*512] -> [4096 o, 512 i]
    wsh = nc.dram_tensor("wsh", [D, ISH], F32, kind="ExternalInput").ap()
    y = nc.dram_tensor("y", [B, D], F32, kind="ExternalOutput").ap()
    # binarized transposed shard per o-half, ot-blocked: [otl][512 i, 512 o]
    N_OTL = OH // OT  # o tiles per half (2)
    wshT_o = [
        nc.dram_tensor(f"wshT_o{h}", [N_OTL, ISH, OT], F16).ap()
        for h in range(N_HALF)
    ]
    # AllGather outputs: [core][otl][512 i, 512 o]
    wT_o = [
        nc.dram_tensor(f"wT_o{h}", [N_CORES, N_OTL, ISH, OT], F16,
                       addr_space="Shared").ap()
        for h in range(N_HALF)
    ]

    with tile.TileContext(nc) as tc:
      for _rep in range(repeat):
        with (
            tc.tile_pool(name="const", bufs=1) as const,
            tc.tile_pool(name="prep", bufs=3) as prep,
            tc.tile_pool(name="xres", bufs=1) as xres,
            tc.tile_pool(name="wres", bufs=1) as wres,
            tc.tile_pool(name="wmov", bufs=6) as wmov,
            tc.tile_pool(name="drain", bufs=6) as drain,
        ):
            id16 = const.tile([128, 128], F16, tag="id16")
            make_identity(nc, id16[:])

            # transposed w shard staging, one slab: itl block at
            # [:, itl*D : (itl+1)*D] holds [128 i, 4096 o]
            wTs = wres.tile([128, N_ITL * D], F16, tag="wts", name="wts")
            # resident transposed x: quarter q: [128 i, 8 itq x 1024 b]
            xthi = [
                xres.tile([128, N_ITQ * B], F16, tag=f"xthi{q}",
                          name=f"xthi{q}")
                for q in range(XQ)
            ]

            if not include_prep:
                for q in range(XQ):
                    nc.gpsimd.memset(xthi[q][:], 0.0)
                nc.gpsimd.memset(wTs[:], 0.0)

            wTs3 = wTs[:].rearrange("p (i o) -> p i o", i=N_ITL)

            with tc.tile_pool(name="tpsum", bufs=3, space="PSUM") as tpsum:
                for h in range(N_HALF if include_prep else 0):
                    # ---- w o-half h: binarize + PE transpose + stage ----
                    for rtl in range(OH // 128):
                        rt = h * (OH // 128) + rtl
                        wa = prep.tile([128, ISH], F32, tag="w_f32_a")
                        eng = nc.scalar if rtl % 2 == 0 else nc.sync
                        eng.dma_start(wa[:], wsh[bass.ts(rt, 128), :])
                        w01 = prep.tile([128, ISH], F32, tag="w_f32_b")
                        nc.vector.tensor_scalar(
                            w01[:], wa[:], BIN_THRESH, None,
                            mybir.AluOpType.is_gt)
                        wb = prep.tile([128, ISH], F16, tag="w_f16_a")
                        nc.vector.tensor_scalar(
                            wb[:], w01[:], 2.0, -1.0,
                            mybir.AluOpType.mult, mybir.AluOpType.add)
                        tw = tpsum.tile([128, N_ITL * 128], F16, tag="tw")
                        for itl in range(N_ITL):
                            nc.tensor.transpose(
                                tw[:, bass.ts(itl, 128)],
                                wb[:, bass.ts(itl, 128)], id16[:])
                        # single strided drain: itl-block k -> wTs[:, k*D + rt*128]
                        nc.vector.tensor_copy(
                            wTs3[:, :, bass.ts(rt, 128)],
                            tw[:].rearrange("p (i k) -> p i k", i=N_ITL))
                    # stage this half of the transposed shard (SWDGE)
                    for itl in range(N_ITL):
                        for otl in range(OH // OT):
                            nc.gpsimd.dma_start(
                                wshT_o[h][otl, bass.ts(itl, 128), :],
                                wTs[:, bass.ds(itl * D + h * OH + otl * OT,
                                               OT)])
                    if include_ag:
                        nc.gpsimd.collective_compute(
                            "AllGather",
                            mybir.AluOpType.bypass,
                            replica_groups=[list(range(N_CORES))],
                            ins=[wshT_o[h][:]],
                            outs=[wT_o[h][:]],
                        )

                    # ---- x i-quarter q=h -> fp16 transposed resident ----
                    q = h
                    xthi3 = xthi[q][:].rearrange("p (i b) -> p i b", i=N_ITQ)
                    for bt in range(N_BT):
                        xa = prep.tile([128, QW], F32, tag="x_f32_a")
                        eng = nc.sync if bt % 2 == 0 else nc.scalar
                        eng.dma_start(
                            xa[:], x[bass.ts(bt, BT), bass.ts(q, QW)])
                        xc = prep.tile([128, QW], F16, tag="x_f16_a")
                        nc.vector.tensor_copy(xc[:], xa[:])
                        for g in range(2):  # two groups of 4 i-tiles
                            tx = tpsum.tile([128, 512], F16, tag="tx")
                            for k in range(4):
                                itq = g * 4 + k
                                nc.tensor.transpose(
                                    tx[:, bass.ts(k, 128)],
                                    xc[:, bass.ts(itq, 128)], id16[:])
                            nc.vector.tensor_copy(
                                xthi3[:, bass.ds(g * 4, 4),
                                      bass.ds(bt * BT, BT)],
                                tx[:].rearrange("p (i k) -> p i k", i=4))

            # ---- matmul phase: w streamed once ----
            with tc.tile_pool(name="psum", bufs=8, space="PSUM") as psum:
                for ot in range(N_OT if include_c else 0):
                    h, otl = divmod(ot, N_OT // N_HALF)
                    pts = []
                    for bt in range(N_BT):
                        pt = psum.tile([128, OT], F32, tag="acc")
                        pts.append(pt)
                    for it in range(N_IT):
                        blk, itl = divmod(it, N_ITL)
                        wt = wmov.tile([128, OT], F16, tag="wmov")
                        eng = nc.sync if it % 2 == 0 else nc.scalar
                        eng.dma_start(
                            wt[:],
                            wT_o[h][blk, otl, bass.ts(itl, IT), :])
                        q, itq = divmod(it, N_ITQ)
                        for bt in range(N_BT):
                            nc.tensor.matmul(
                                pts[bt][:],
                                xthi[q][:, bass.ds(itq * B + bt * BT, BT)],
                                wt[:],
                                start=(it == 0), stop=(it == N_IT - 1))
                    for bt in range(N_BT):
                        st = drain.tile([128, OT], F32, tag="drain")
                        nc.vector.tensor_copy(st[:], pts[bt][:])
                        nc.gpsimd.dma_start(
                            y[bass.ts(bt, BT), bass.ts(ot, OT)], st[:])

    nc.finalize()
    return nc


def _get_nc():
    if "nc" not in _CACHED:
        _CACHED["nc"] = _build()
    return _CACHED["nc"]


def build_nc(repeat=1, **kw):
    return _build(repeat=repeat, **kw)


def run(x, weight, **run_kwargs):
    nc = _get_nc()
    x = np.ascontiguousarray(x, dtype=np.float32)
    weight = np.ascontiguousarray(weight, dtype=np.float32)
    in_maps = [
        {"x": x[c * B:(c + 1) * B],
         "wsh": np.ascontiguousarray(weight[:, c * ISH:(c + 1) * ISH])}
        for c in range(N_CORES)
    ]
    res = run_bass_kernel_spmd(nc, in_maps, list(range(N_CORES)), **run_kwargs)
    out = np.concatenate([res.results[c]["y"] for c in range(N_CORES)], axis=0)
    return out, res


def kernel(x, weight):
    out, _ = run(x, weight)
    return out


# revision 3
# speedup vs baseline: 1.1169x; 1.1169x over previous
"""Trainium2 Bass kernel (v8: deeper buffers + single-op Sign binarize on ACT) for nn_BinaryDense: y = x @ binarize(w).T

x: [8192, 4096] f32, weight: [4096, 4096] f32 -> y: [8192, 4096] f32.

binarize(w) = +1 if fp32(w + 1.0) > 1.0 else -1, i.e. w > 2**-24 -> +1
(reference: round-half-even(clip((w+1)/2, 0, 1)) * 2 - 1 with H=1).

Strategy (8 cores), v5 — v4 architecture (PE transposes + pipelined fp16
AllGather, interleaved w-half/x-quarter prep, SWDGE stores) with the
prep critical path tightened:

  - all PE transposes run in fp16 (x is DVE-cast to fp16 before its
    transpose; fp32 transpose-mode is 2 cycles/row, fp16 is 1).
  - transposes write groups of four 128x128 tiles into one [128, 512]
    PSUM tile, drained by a SINGLE strided DVE copy (3D access pattern)
    instead of four small ones — 96 PSUM-drain copies total instead of
    384, removing DVE instruction-overhead pacing from the prep span.
  - w^T staging is one [128, 16K] slab (itl-blocked) so the grouped
    copies land with a strided AP; AllGather staging stores slice it.
  - matmul phase unchanged from v4: per 512-col o-tile, stream w^T
    i-tiles on alternating HWDGE queues, one fp16 matmul pass into fp32
    PSUM (8 banks = 8 row tiles), drain via DVE + SWDGE store.
"""

import numpy as np

import concourse.bass as bass
import concourse.tile as tile
from concourse import bacc, mybir
from concourse.bass_utils import run_bass_kernel_spmd
from concourse.masks import make_identity

N_CORES = 8
B = 1024            # rows of x per core
D = 4096            # in/out features
ISH = D // N_CORES  # 512, i-columns of w per core
BT = 128            # b tile (psum partition)
OT = 512            # o tile (psum free / one bank)
IT = 128            # contraction tile (partitions)
N_BT = B // BT      # 8
N_OT = D // OT      # 8
N_IT = D // IT      # 32
N_HALF = 4          # AllGather pipeline depth (o-range splits)
OH = D // N_HALF    # 1024 o-rows per collective
XQ = 4              # x i-quarters
QW = D // XQ        # 1024 i-cols per quarter
N_ITQ = N_IT // XQ  # 8 i-tiles per quarter
N_ITL = ISH // 128  # 4 i-tile rows of the w shard

F32 = mybir.dt.float32
F16 = mybir.dt.float16

BIN_THRESH = float(2.0 ** -24)

_CACHED = {}


def _build(repeat=1, include_ag=True, include_c=True, include_prep=True):
    nc = bacc.Bacc("TRN2", target_bir_lowering=False, debug=False,
                   num_devices=N_CORES)
    x = nc.dram_tensor("x", [B, D], F32, kind="ExternalInput").ap()
    # w columns shard: w[:, c*512:(c+1)*512] -> [4096 o, 512 i]
    wsh = nc.dram_tensor("wsh", [D, ISH], F32, kind="ExternalInput").ap()
    y = nc.dram_tensor("y", [B, D], F32, kind="ExternalOutput").ap()
    # binarized transposed shard per o-half, ot-blocked: [otl][512 i, 512 o]
    N_OTL = OH // OT  # o tiles per half (2)
    wshT_o = [
        nc.dram_tensor(f"wshT_o{h}", [N_OTL, ISH, OT], F16).ap()
        for h in range(N_HALF)
    ]
    # AllGather outputs: [core][otl][512 i, 512 o]
    wT_o = [
        nc.dram_tensor(f"wT_o{h}", [N_CORES, N_OTL, ISH, OT], F16,
                       addr_space="Shared").ap()
        for h in range(N_HALF)
    ]

    with tile.TileContext(nc) as tc:
      for _rep in range(repeat):
        with (
            tc.tile_pool(name="const", bufs=1) as const,
            tc.tile_pool(name="prep", bufs=4) as prep,
            tc.tile_pool(name="xres", bufs=1) as xres,
            tc.tile_pool(name="wres", bufs=1) as wres,
            tc.tile_pool(name="wmov", bufs=8) as wmov,
            tc.tile_pool(name="drain", bufs=6) as drain,
        ):
            id16 = const.tile([128, 128], F16, tag="id16")
            make_identity(nc, id16[:])
            bneg = const.tile([128, 1], F32, tag="bneg")
            nc.gpsimd.memset(bneg[:], -BIN_THRESH)

            # transposed w shard staging, one slab: itl block at
            # [:, itl*D : (itl+1)*D] holds [128 i, 4096 o]
            wTs = wres.tile([128, N_ITL * D], F16, tag="wts", name="wts")
            # resident transposed x: quarter q: [128 i, 8 itq x 1024 b]
            xthi = [
                xres.tile([128, N_ITQ * B], F16, tag=f"xthi{q}",
                          name=f"xthi{q}")
                for q in range(XQ)
            ]

            if not include_prep:
                for q in range(XQ):
                    nc.gpsimd.memset(xthi[q][:], 0.0)
                nc.gpsimd.memset(wTs[:], 0.0)

            wTs3 = wTs[:].rearrange("p (i o) -> p i o", i=N_ITL)

            with tc.tile_pool(name="tpsum", bufs=4, space="PSUM") as tpsum:
                for h in range(N_HALF if include_prep else 0):
                    # ---- w o-half h: binarize + PE transpose + stage ----
                    for rtl in range(OH // 128):
                        rt = h * (OH // 128) + rtl
                        wa = prep.tile([128, ISH], F32, tag="w_f32_a")
                        eng = nc.scalar if rtl % 2 == 0 else nc.sync
                        eng.dma_start(wa[:], wsh[bass.ts(rt, 128), :])
                        wb = prep.tile([128, ISH], F16, tag="w_f16_a")
                        nc.scalar.activation(
                            wb[:], wa[:], mybir.ActivationFunctionType.Sign,
                            bias=bneg[:])
                        tw = tpsum.tile([128, N_ITL * 128], F16, tag="tw")
                        for itl in range(N_ITL):
                            nc.tensor.transpose(
                                tw[:, bass.ts(itl, 128)],
                                wb[:, bass.ts(itl, 128)], id16[:])
                        # single strided drain: itl-block k -> wTs[:, k*D + rt*128]
                        nc.vector.tensor_copy(
                            wTs3[:, :, bass.ts(rt, 128)],
                            tw[:].rearrange("p (i k) -> p i k", i=N_ITL))
                    # stage this half of the transposed shard (SWDGE)
                    for itl in range(N_ITL):
                        for otl in range(OH // OT):
                            nc.gpsimd.dma_start(
                                wshT_o[h][otl, bass.ts(itl, 128), :],
                                wTs[:, bass.ds(itl * D + h * OH + otl * OT,
                                               OT)])
                    if include_ag:
                        nc.gpsimd.collective_compute(
                            "AllGather",
                            mybir.AluOpType.bypass,
                            replica_groups=[list(range(N_CORES))],
                            ins=[wshT_o[h][:]],
                            outs=[wT_o[h][:]],
                        )

                    # ---- x i-quarter q=h -> fp16 transposed resident ----
                    q = h
                    xthi3 = xthi[q][:].rearrange("p (i b) -> p i b", i=N_ITQ)
                    for bt in range(N_BT):
                        xa = prep.tile([128, QW], F32, tag="x_f32_a")
                        eng = nc.sync if bt % 2 == 0 else nc.scalar
                        eng.dma_start(
                            xa[:], x[bass.ts(bt, BT), bass.ts(q, QW)])
                        xc = prep.tile([128, QW], F16, tag="x_f16_a")
                        nc.vector.tensor_copy(xc[:], xa[:])
                        for g in range(2):  # two groups of 4 i-tiles
                            tx = tpsum.tile([128, 512], F16, tag="tx")
                            for k in range(4):
                                itq = g * 4 + k
                                nc.tensor.transpose(
                                    tx[:, bass.ts(k, 128)],
                                    xc[:, bass.ts(itq, 128)], id16[:])
                            nc.vector.tensor_copy(
                                xthi3[:, bass.ds(g * 4, 4),
                                      bass.ds(bt * BT, BT)],
                                tx[:].rearrange("p (i k) -> p i k", i=4))

            # ---- matmul phase: w streamed once ----
            with tc.tile_pool(name="psum", bufs=8, space="PSUM") as psum:
                for ot in range(N_OT if include_c else 0):
                    h, otl = divmod(ot, N_OT // N_HALF)
                    pts = []
                    for bt in range(N_BT):
                        pt = psum.tile([128, OT], F32, tag="acc")
                        pts.append(pt)
                    for it in range(N_IT):
                        blk, itl = divmod(it, N_ITL)
                        wt = wmov.tile([128, OT], F16, tag="wmov")
                        eng = nc.sync if it % 2 == 0 else nc.scalar
                        eng.dma_start(
                            wt[:],
                            wT_o[h][blk, otl, bass.ts(itl, IT), :])
                        q, itq = divmod(it, N_ITQ)
                        for bt in range(N_BT):
                            nc.tensor.matmul(
                                pts[bt][:],
                                xthi[q][:, bass.ds(itq * B + bt * BT, BT)],
                                wt[:],
                                start=(it == 0), stop=(it == N_IT - 1))
                    for bt in range(N_BT):
                        st = drain.tile([128, OT], F32, tag="drain")
                        nc.vector.tensor_copy(st[:], pts[bt][:])
                        nc.gpsimd.dma_start(
                            y[bass.ts(bt, BT), bass.ts(ot, OT)], st[:])

    nc.finalize()
    return nc


def _get_nc():
    if "nc" not in _CACHED:
        _CACHED["nc"] = _build()
    return _CACHED["nc"]


def build_nc(repeat=1, **kw):
    return _build(repeat=repeat, **kw)


def run(x, weight, **run_kwargs):
    nc = _get_nc()
    x = np.ascontiguousarray(x, dtype=np.float32)
    weight = np.ascontiguousarray(weight, dtype=np.float32)
    in_maps = [
        {"x": x[c * B:(c + 1) * B],
         "wsh": np.ascontiguousarray(weight[:, c * ISH:(c + 1) * ISH])}
        for c in range(N_CORES)
    ]
    res = run_bass_kernel_spmd(nc, in_maps, list(range(N_CORES)), **run_kwargs)
    out = np.concatenate([res.results[c]["y"] for c in range(N_CORES)], axis=0)
    return out, res


def kernel(x, weight):
    out, _ = run(x, weight)
    return out


# revision 4
# speedup vs baseline: 1.1330x; 1.0144x over previous
"""Trainium2 Bass kernel (v9: bufs=6 prep pipelining + single-op Sign binarize) for nn_BinaryDense: y = x @ binarize(w).T

x: [8192, 4096] f32, weight: [4096, 4096] f32 -> y: [8192, 4096] f32.

binarize(w) = +1 if fp32(w + 1.0) > 1.0 else -1, i.e. w > 2**-24 -> +1
(reference: round-half-even(clip((w+1)/2, 0, 1)) * 2 - 1 with H=1).

Strategy (8 cores), v5 — v4 architecture (PE transposes + pipelined fp16
AllGather, interleaved w-half/x-quarter prep, SWDGE stores) with the
prep critical path tightened:

  - all PE transposes run in fp16 (x is DVE-cast to fp16 before its
    transpose; fp32 transpose-mode is 2 cycles/row, fp16 is 1).
  - transposes write groups of four 128x128 tiles into one [128, 512]
    PSUM tile, drained by a SINGLE strided DVE copy (3D access pattern)
    instead of four small ones — 96 PSUM-drain copies total instead of
    384, removing DVE instruction-overhead pacing from the prep span.
  - w^T staging is one [128, 16K] slab (itl-blocked) so the grouped
    copies land with a strided AP; AllGather staging stores slice it.
  - matmul phase unchanged from v4: per 512-col o-tile, stream w^T
    i-tiles on alternating HWDGE queues, one fp16 matmul pass into fp32
    PSUM (8 banks = 8 row tiles), drain via DVE + SWDGE store.
"""

import numpy as np

import concourse.bass as bass
import concourse.tile as tile
from concourse import bacc, mybir
from concourse.bass_utils import run_bass_kernel_spmd
from concourse.masks import make_identity

N_CORES = 8
B = 1024            # rows of x per core
D = 4096            # in/out features
ISH = D // N_CORES  # 512, i-columns of w per core
BT = 128            # b tile (psum partition)
OT = 512            # o tile (psum free / one bank)
IT = 128            # contraction tile (partitions)
N_BT = B // BT      # 8
N_OT = D // OT      # 8
N_IT = D // IT      # 32
N_HALF = 4          # AllGather pipeline depth (o-range splits)
OH = D // N_HALF    # 1024 o-rows per collective
XQ = 4              # x i-quarters
QW = D // XQ        # 1024 i-cols per quarter
N_ITQ = N_IT // XQ  # 8 i-tiles per quarter
N_ITL = ISH // 128  # 4 i-tile rows of the w shard

F32 = mybir.dt.float32
F16 = mybir.dt.float16

BIN_THRESH = float(2.0 ** -24)

_CACHED = {}


def _build(repeat=1, include_ag=True, include_c=True, include_prep=True):
    nc = bacc.Bacc("TRN2", target_bir_lowering=False, debug=False,
                   num_devices=N_CORES)
    x = nc.dram_tensor("x", [B, D], F32, kind="ExternalInput").ap()
    # w columns shard: w[:, c*512:(c+1)*512] -> [4096 o, 512 i]
    wsh = nc.dram_tensor("wsh", [D, ISH], F32, kind="ExternalInput").ap()
    y = nc.dram_tensor("y", [B, D], F32, kind="ExternalOutput").ap()
    # binarized transposed shard per o-half, ot-blocked: [otl][512 i, 512 o]
    N_OTL = OH // OT  # o tiles per half (2)
    wshT_o = [
        nc.dram_tensor(f"wshT_o{h}", [N_OTL, ISH, OT], F16).ap()
        for h in range(N_HALF)
    ]
    # AllGather outputs: [core][otl][512 i, 512 o]
    wT_o = [
        nc.dram_tensor(f"wT_o{h}", [N_CORES, N_OTL, ISH, OT], F16,
                       addr_space="Shared").ap()
        for h in range(N_HALF)
    ]

    with tile.TileContext(nc) as tc:
      for _rep in range(repeat):
        with (
            tc.tile_pool(name="const", bufs=1) as const,
            tc.tile_pool(name="prep", bufs=6) as prep,
            tc.tile_pool(name="xres", bufs=1) as xres,
            tc.tile_pool(name="wres", bufs=1) as wres,
            tc.tile_pool(name="wmov", bufs=8) as wmov,
            tc.tile_pool(name="drain", bufs=6) as drain,
        ):
            id16 = const.tile([128, 128], F16, tag="id16")
            make_identity(nc, id16[:])
            bneg = const.tile([128, 1], F32, tag="bneg")
            nc.gpsimd.memset(bneg[:], -BIN_THRESH)

            # transposed w shard staging, one slab: itl block at
            # [:, itl*D : (itl+1)*D] holds [128 i, 4096 o]
            wTs = wres.tile([128, N_ITL * D], F16, tag="wts", name="wts")
            # resident transposed x: quarter q: [128 i, 8 itq x 1024 b]
            xthi = [
                xres.tile([128, N_ITQ * B], F16, tag=f"xthi{q}",
                          name=f"xthi{q}")
                for q in range(XQ)
            ]

            if not include_prep:
                for q in range(XQ):
                    nc.gpsimd.memset(xthi[q][:], 0.0)
                nc.gpsimd.memset(wTs[:], 0.0)

            wTs3 = wTs[:].rearrange("p (i o) -> p i o", i=N_ITL)

            with tc.tile_pool(name="tpsum", bufs=4, space="PSUM") as tpsum:
                for h in range(N_HALF if include_prep else 0):
                    # ---- w o-half h: binarize + PE transpose + stage ----
                    for rtl in range(OH // 128):
                        rt = h * (OH // 128) + rtl
                        wa = prep.tile([128, ISH], F32, tag="w_f32_a")
                        eng = nc.scalar if rtl % 2 == 0 else nc.sync
                        eng.dma_start(wa[:], wsh[bass.ts(rt, 128), :])
                        wb = prep.tile([128, ISH], F16, tag="w_f16_a")
                        nc.scalar.activation(
                            wb[:], wa[:], mybir.ActivationFunctionType.Sign,
                            bias=bneg[:])
                        tw = tpsum.tile([128, N_ITL * 128], F16, tag="tw")
                        for itl in range(N_ITL):
                            nc.tensor.transpose(
                                tw[:, bass.ts(itl, 128)],
                                wb[:, bass.ts(itl, 128)], id16[:])
                        # single strided drain: itl-block k -> wTs[:, k*D + rt*128]
                        nc.vector.tensor_copy(
                            wTs3[:, :, bass.ts(rt, 128)],
                            tw[:].rearrange("p (i k) -> p i k", i=N_ITL))
                    # stage this half of the transposed shard (SWDGE)
                    for itl in range(N_ITL):
                        for otl in range(OH // OT):
                            nc.gpsimd.dma_start(
                                wshT_o[h][otl, bass.ts(itl, 128), :],
                                wTs[:, bass.ds(itl * D + h * OH + otl * OT,
                                               OT)])
                    if include_ag:
                        nc.gpsimd.collective_compute(
                            "AllGather",
                            mybir.AluOpType.bypass,
                            replica_groups=[list(range(N_CORES))],
                            ins=[wshT_o[h][:]],
                            outs=[wT_o[h][:]],
                        )

                    # ---- x i-quarter q=h -> fp16 transposed resident ----
                    q = h
                    xthi3 = xthi[q][:].rearrange("p (i b) -> p i b", i=N_ITQ)
                    for bt in range(N_BT):
                        xa = prep.tile([128, QW], F32, tag="x_f32_a")
                        eng = nc.sync if bt % 2 == 0 else nc.scalar
                        eng.dma_start(
                            xa[:], x[bass.ts(bt, BT), bass.ts(q, QW)])
                        xc = prep.tile([128, QW], F16, tag="x_f16_a")
                        nc.vector.tensor_copy(xc[:], xa[:])
                        for g in range(2):  # two groups of 4 i-tiles
                            tx = tpsum.tile([128, 512], F16, tag="tx")
                            for k in range(4):
                                itq = g * 4 + k
                                nc.tensor.transpose(
                                    tx[:, bass.ts(k, 128)],
                                    xc[:, bass.ts(itq, 128)], id16[:])
                            nc.vector.tensor_copy(
                                xthi3[:, bass.ds(g * 4, 4),
                                      bass.ds(bt * BT, BT)],
                                tx[:].rearrange("p (i k) -> p i k", i=4))

            # ---- matmul phase: w streamed once ----
            with tc.tile_pool(name="psum", bufs=8, space="PSUM") as psum:
                for ot in range(N_OT if include_c else 0):
                    h, otl = divmod(ot, N_OT // N_HALF)
                    pts = []
                    for bt in range(N_BT):
                        pt = psum.tile([128, OT], F32, tag="acc")
                        pts.append(pt)
                    for it in range(N_IT):
                        blk, itl = divmod(it, N_ITL)
                        wt = wmov.tile([128, OT], F16, tag="wmov")
                        eng = nc.sync if it % 2 == 0 else nc.scalar
                        eng.dma_start(
                            wt[:],
                            wT_o[h][blk, otl, bass.ts(itl, IT), :])
                        q, itq = divmod(it, N_ITQ)
                        for bt in range(N_BT):
                            nc.tensor.matmul(
                                pts[bt][:],
                                xthi[q][:, bass.ds(itq * B + bt * BT, BT)],
                                wt[:],
                                start=(it == 0), stop=(it == N_IT - 1))
                    for bt in range(N_BT):
                        st = drain.tile([128, OT], F32, tag="drain")
                        nc.vector.tensor_copy(st[:], pts[bt][:])
                        nc.gpsimd.dma_start(
                            y[bass.ts(bt, BT), bass.ts(ot, OT)], st[:])

    nc.finalize()
    return nc


def _get_nc():
    if "nc" not in _CACHED:
        _CACHED["nc"] = _build()
    return _CACHED["nc"]


def build_nc(repeat=1, **kw):
    return _build(repeat=repeat, **kw)


def run(x, weight, **run_kwargs):
    nc = _get_nc()
    x = np.ascontiguousarray(x, dtype=np.float32)
    weight = np.ascontiguousarray(weight, dtype=np.float32)
    in_maps = [
        {"x": x[c * B:(c + 1) * B],
         "wsh": np.ascontiguousarray(weight[:, c * ISH:(c + 1) * ISH])}
        for c in range(N_CORES)
    ]
    res = run_bass_kernel_spmd(nc, in_maps, list(range(N_CORES)), **run_kwargs)
    out = np.concatenate([res.results[c]["y"] for c in range(N_CORES)], axis=0)
    return out, res


def kernel(x, weight):
    out, _ = run(x, weight)
    return out
